# revision 1
# baseline (speedup 1.0000x reference)
"""Complex self-attention (single-head) on 8 Trainium2 NeuronCores.

Problem: y = stack(re, im) of softmax(|q k^H|/sqrt(D)) @ v with complex
q/k/v projections of a complex input x.  B=8, N=1024, D=512, fp32 I/O.

Strategy
--------
Data-parallel over the batch: core c computes batch c entirely locally.

Per-core math (all matmuls fp16 operands, fp32 PSUM accumulation):
  * Host pre-transposes x (-> x^T [D, N]) and ships transposed / negated /
    pre-scaled weight variants, so no on-device transposes are needed.
  * sqrt(1/sqrt(D)) is folded into BOTH Wq and Wk (and bq, bk) so the
    score scale comes out exactly right with zero device work.
  * Projections accumulate complex parts directly in PSUM:
      q^T = Wq^T.T @ x^T   (lhsT = Wq^T chunk, rhs = x^T chunk)
      v   = x^T.T @ Wv^T   (lhsT = x^T chunk, rhs = Wv^T chunk)
  * Scores are computed TRANSPOSED, s^T[m, n] = sum_e k^T[e,m] q^T[e,n],
    so that E = exp(|s|) lands in [m, n] layout, which is exactly the
    lhsT layout the att@v matmul wants.  Softmax then needs NO max, NO
    transpose and NO vector reductions:
      - exp without max-subtraction is safe (|s| <= ~20); a constant
        EXP_SHIFT keeps exp() within fp16 range, and cancels in U/Z.
      - Z[n] = sum_m E[m,n] comes from a matmul against a ones column.
      - w = (E^T.T @ v) * (1/Z) with a per-partition scalar multiply.
"""

from contextlib import ExitStack

import numpy as np

import concourse.bass as bass
import concourse.mybir as mybir
import concourse.tile as tile
from concourse import bacc
from concourse.bass_utils import run_bass_kernel_spmd

B, N, D = 8, 1024, 512
P = 128
KC = D // P          # 4 contraction chunks of 128
MC = N // P          # 8 row chunks of 128
NH = 2               # halves of N (free dim <= 512 per matmul)
NCORES = 8
EXP_SHIFT = 11.0     # exp(|s| - SHIFT): keeps E in fp16 range; cancels in U/Z

f16 = mybir.dt.float16
f32 = mybir.dt.float32
AF = mybir.ActivationFunctionType


def emit(tc, ctx, nc, xr_d, xi_d, w_d, bqk_d, out_d):
    singles = ctx.enter_context(tc.tile_pool(name="singles", bufs=1))
    ps = ctx.enter_context(tc.tile_pool(name="ps", bufs=7, space="PSUM"))
    psz = ctx.enter_context(tc.tile_pool(name="psz", bufs=1, space="PSUM"))
    tmp = ctx.enter_context(tc.tile_pool(name="tmp", bufs=4))
    outp = ctx.enter_context(tc.tile_pool(name="outp", bufs=4))

    # ---- inputs to SBUF -------------------------------------------------
    # All loads on the SP HWDGE ring (FIFO), split per contraction-chunk and
    # interleaved in exact first-use order so matmuls start early.
    xr_sb = singles.tile([P, KC, N], f16)
    xi_sb = singles.tile([P, KC, N], f16)
    xs_sb = singles.tile([P, KC, N], f16)    # x_re + x_im (computed on DVE)
    xr_r = xr_d.rearrange("(c p) n -> p c n", p=P)
    xi_r = xi_d.rearrange("(c p) n -> p c n", p=P)
    w_sb = singles.tile([P, 9, KC, D], f16)
    w_r = [w_d[t].rearrange("(c p) e -> p c e", p=P) for t in range(9)]

    bqk_sb = singles.tile([P, 4, KC], f32)
    # single HWDGE ring (SP): interleave weight/x chunks in exact first-use
    # order so the projection matmuls are never waiting on a later transfer.
    for kc in range(KC):
        nc.sync.dma_start(out=w_sb[:, 0, kc], in_=w_r[0][:, kc])
        nc.sync.dma_start(out=xr_sb[:, kc], in_=xr_r[:, kc])
        if kc == 0:  # tiny; after the critical first pair
            nc.sync.dma_start(out=bqk_sb, in_=bqk_d)
    for kc in range(KC):
        nc.sync.dma_start(out=w_sb[:, 1, kc], in_=w_r[1][:, kc])
        nc.sync.dma_start(out=xi_sb[:, kc], in_=xi_r[:, kc])
        nc.vector.tensor_add(xs_sb[:, kc], xr_sb[:, kc], xi_sb[:, kc])
    for t in (2, 3, 4, 5, 6, 7, 8):
        for kc in range(KC):
            nc.sync.dma_start(out=w_sb[:, t, kc], in_=w_r[t][:, kc])

    ones_m = singles.tile([P, 1], f16)
    nc.vector.memset(ones_m, 1.0)
    shift_sb = singles.tile([P, 1], f32)
    nc.vector.memset(shift_sb, -EXP_SHIFT)

    # ---- persistent intermediates --------------------------------------
    # qk_sb slots: 0 qr^T, 1 qi^T, 2 -qi^T, 3 kr^T, 4 ki^T   (each [e, n])
    qk_sb = singles.tile([P, 5, KC, N], f16)
    v_sb = singles.tile([P, 2, MC, D], f16)      # v[, m-chunk, d] re/im
    et_sb = singles.tile([P, MC, N], f16)        # E^T[m, n] = exp(|s|-SHIFT)

    # ---- q/k projections (Karatsuba: 3 products per complex matmul) -----
    # T1 = Wr x_re, T2 = Wi x_im, T3 = Wsum x_sum;
    # re = T1 - T2 + b_r;  im = T3 - T1 - T2 + b_i.
    # (w slots r/i/sum, bias slots b_r / b_r+b_i, dst slots re/im/neg-im)
    qk_spec = [
        ((0, 1, 2), (0, 1), (0, 1, 2)),      # q (also writes -qi)
        ((3, 4, 5), (2, 3), (3, 4, None)),   # k
    ]
    t1bp = ctx.enter_context(tc.tile_pool(name="t1bp", bufs=9))
    t12p = ctx.enter_context(tc.tile_pool(name="t12p", bufs=9))
    tiles = [(ec, nh) for ec in range(KC) for nh in range(NH)]
    for (w_r, w_i, w_s), (b_r, b_s), (d_r, d_i, d_n) in qk_spec:
        # phase A: T1 = Wr x_re -> t1b = T1 + b_r   (only needs Wr + x_re).
        # For the very first pair, run kc-major over groups of 4 tiles so PE
        # consumes (w chunk, x chunk) pairs in DMA arrival order.
        t1bs, t12s = {}, {}
        for g0 in range(0, len(tiles), 4):
            grp = tiles[g0:g0 + 4]
            pts = [ps.tile([P, 512], f32, tag="b", name="pt") for _ in grp]
            for kc in range(KC):
                for pt, (ec, nh) in zip(pts, grp):
                    nc.tensor.matmul(
                        pt, lhsT=w_sb[:, w_r, kc, ec * P:(ec + 1) * P],
                        rhs=xr_sb[:, kc, nh * 512:nh * 512 + 512],
                        start=(kc == 0), stop=(kc == KC - 1),
                    )
            for pt, (ec, nh) in zip(pts, grp):
                t1b = t1bp.tile([P, 512], f32, tag="t1b", name="t1b")
                nc.scalar.activation(
                    out=t1b, in_=pt, func=AF.Identity,
                    bias=bqk_sb[:, b_r, ec:ec + 1],
                )
                t1bs[ec, nh] = t1b
        # phase B: T2 = Wi x_im -> re = t1b - T2 ; t12b = T2 + t1b
        for ec, nh in tiles:
            n0, e0 = nh * 512, ec * P
            pt = ps.tile([P, 512], f32, tag="b", name="pt")
            for kc in range(KC):
                nc.tensor.matmul(
                    pt, lhsT=w_sb[:, w_i, kc, e0:e0 + P],
                    rhs=xi_sb[:, kc, n0:n0 + 512],
                    start=(kc == 0), stop=(kc == KC - 1),
                )
            nc.vector.scalar_tensor_tensor(
                out=qk_sb[:, d_r, ec, n0:n0 + 512],
                in0=pt, scalar=-1.0, in1=t1bs[ec, nh],
                op0=mybir.AluOpType.mult, op1=mybir.AluOpType.add,
            )
            t12b = t12p.tile([P, 512], f32, tag="t12b", name="t12b")
            nc.vector.tensor_add(t12b, pt, t1bs[ec, nh])
            t12s[ec, nh] = t12b
        # phase C: T3 = Wsum x_sum -> im = (T3 + b_sum) - t12b  (and -im)
        for ec, nh in tiles:
            n0, e0 = nh * 512, ec * P
            pt = ps.tile([P, 512], f32, tag="b", name="pt")
            for kc in range(KC):
                nc.tensor.matmul(
                    pt, lhsT=w_sb[:, w_s, kc, e0:e0 + P],
                    rhs=xs_sb[:, kc, n0:n0 + 512],
                    start=(kc == 0), stop=(kc == KC - 1),
                )
            nc.vector.scalar_tensor_tensor(
                out=qk_sb[:, d_i, ec, n0:n0 + 512],
                in0=pt, scalar=bqk_sb[:, b_s, ec:ec + 1], in1=t12s[ec, nh],
                op0=mybir.AluOpType.add, op1=mybir.AluOpType.subtract,
            )
            if d_n is not None:  # -qi from qi (fast fp16 sbuf pass)
                nc.vector.tensor_scalar(
                    qk_sb[:, d_n, ec, n0:n0 + 512],
                    qk_sb[:, d_i, ec, n0:n0 + 512],
                    -1.0, None, mybir.AluOpType.mult,
                )

    # ---- v projection (Karatsuba; bias deferred to after attention -------
    # since softmax rows sum to 1, w = U/Z + bv exactly) ------------------
    for mc in range(MC):
        m0 = mc * P
        prods = []
        for xs, wi in ((xr_sb, 6), (xi_sb, 7), (xs_sb, 8)):
            pt = ps.tile([P, 512], f32, tag="b", name="pt")
            for kc in range(KC):
                nc.tensor.matmul(
                    pt,
                    lhsT=xs[:, kc, m0:m0 + P],
                    rhs=w_sb[:, wi, kc, :],
                    start=(kc == 0),
                    stop=(kc == KC - 1),
                )
            prods.append(pt)
        t1, t2, t3 = prods
        t1s = t1bp.tile([P, 512], f32, tag="t1b", name="t1s")
        nc.scalar.activation(out=t1s, in_=t1, func=AF.Copy)
        nc.vector.scalar_tensor_tensor(
            out=v_sb[:, 0, mc, :], in0=t2, scalar=-1.0, in1=t1s,
            op0=mybir.AluOpType.mult, op1=mybir.AluOpType.add,
        )
        t12 = t12p.tile([P, 512], f32, tag="t12b", name="t12")
        nc.vector.tensor_add(t12, t2, t1s)
        nc.vector.scalar_tensor_tensor(
            out=v_sb[:, 1, mc, :], in0=t3, scalar=0.0, in1=t12,
            op0=mybir.AluOpType.bypass, op1=mybir.AluOpType.subtract,
        )

    # ---- scores + softmax numerator / AV, half by half ------------------
    def scores_half(nh):
        n0 = nh * 512
        for mc in range(MC):
            m0 = mc * P
            rt = ps.tile([P, 512], f32, tag="b", name="rt")
            it = ps.tile([P, 512], f32, tag="b", name="it")
            for out_t, pairs in ((rt, ((3, 0), (4, 2))), (it, ((3, 1), (4, 0)))):
                idx = 0
                for kt, qt in pairs:
                    for ec in range(KC):
                        nc.tensor.matmul(
                            out_t,
                            lhsT=qk_sb[:, kt, ec, m0:m0 + P],
                            rhs=qk_sb[:, qt, ec, n0:n0 + 512],
                            start=(idx == 0),
                            stop=(idx == 7),
                        )
                        idx += 1
            t1 = tmp.tile([P, 512], f32, tag="sq", name="t1")
            nc.scalar.activation(out=t1, in_=rt, func=AF.Square)
            t2 = tmp.tile([P, 512], f32, tag="sq", name="t2")
            nc.scalar.activation(out=t2, in_=it, func=AF.Square)
            u = tmp.tile([P, 512], f32, tag="u", name="u")
            nc.vector.tensor_add(u, t1, t2)
            a = tmp.tile([P, 512], f32, tag="a", name="a")
            nc.scalar.activation(out=a, in_=u, func=AF.Sqrt)
            nc.scalar.activation(
                out=et_sb[:, mc, n0:n0 + 512], in_=a, func=AF.Exp,
                bias=shift_sb,
            )

    def av_half(nh):
        for g in range(nh * 4, nh * 4 + 4):
            last = g == 7
            zp = psz.tile([P, 1], f32, tag="z", name="zp")
            if last:  # Z first so 1/Z is ready while U is still accumulating
                for mc in range(MC):
                    nc.tensor.matmul(
                        zp, lhsT=et_sb[:, mc, g * P:(g + 1) * P], rhs=ones_m,
                        start=mc == 0, stop=mc == MC - 1,
                    )
                zr = tmp.tile([P, 1], f32, tag="zr", name="zr")
                nc.vector.reciprocal(zr, zp)
            for h0, hw in ((0, 512),):
                ur = ps.tile([P, 512], f32, tag="b", name="ur")
                ui = ps.tile([P, 512], f32, tag="b", name="ui")
                for mc in range(MC):
                    lh = et_sb[:, mc, g * P:(g + 1) * P]
                    st, sp = mc == 0, mc == MC - 1
                    nc.tensor.matmul(ur[:, :hw], lhsT=lh,
                                     rhs=v_sb[:, 0, mc, h0:h0 + hw], start=st, stop=sp)
                    nc.tensor.matmul(ui[:, :hw], lhsT=lh,
                                     rhs=v_sb[:, 1, mc, h0:h0 + hw], start=st, stop=sp)
                    if not last:
                        nc.tensor.matmul(zp, lhsT=lh, rhs=ones_m, start=st, stop=sp)
                if not last:
                    zr = tmp.tile([P, 1], f32, tag="zr", name="zr")
                    nc.vector.reciprocal(zr, zp)
                # w = U * (1/Z); the v bias is added on the host (exact,
                # since sum(att) = 1). re on DVE, im on ACT: the two chains
                # of the final chunk run in parallel at the tail.
                o0 = outp.tile([P, 512], f16, tag="o", name="o0")
                nc.vector.tensor_scalar_mul(o0[:, :hw], ur[:, :hw], zr)
                nc.sync.dma_start(
                    out=out_d[0, g * P:(g + 1) * P, h0:h0 + hw], in_=o0[:, :hw])
                o1 = outp.tile([P, 512], f16, tag="o", name="o1")
                nc.scalar.activation(out=o1[:, :hw], in_=ui[:, :hw],
                                     func=AF.Copy, scale=zr)
                nc.scalar.dma_start(
                    out=out_d[1, g * P:(g + 1) * P, h0:h0 + hw], in_=o1[:, :hw])

    scores_half(0)
    av_half(0)
    scores_half(1)
    av_half(1)


def build_nc():
    nc = bacc.Bacc("TRN2", target_bir_lowering=False, debug=False)
    xr_d = nc.dram_tensor("xrT", [D, N], f16, kind="ExternalInput").ap()
    xi_d = nc.dram_tensor("xiT", [D, N], f16, kind="ExternalInput").ap()
    w_d = nc.dram_tensor("w9", [9, D, D], f16, kind="ExternalInput").ap()
    bqk_d = nc.dram_tensor("bqk", [P, 4, KC], f32, kind="ExternalInput").ap()
    out_d = nc.dram_tensor("out", [2, N, D], f16, kind="ExternalOutput").ap()
    with tile.TileContext(nc) as tc, ExitStack() as ctx:
        emit(tc, ctx, nc, xr_d, xi_d, w_d, bqk_d, out_d)
    nc.compile()
    return nc


def make_in_maps(inputs):
    sc = float((1.0 / np.sqrt(D)) ** 0.5)

    def t16(a, s=1.0):
        return np.ascontiguousarray(a.T * s).astype(np.float16)

    # w slots: q r/i/sum (scaled), k r/i/sum (scaled), v r / i / -i
    w9 = np.stack([
        t16(inputs["Wq_re"], sc), t16(inputs["Wq_im"], sc),
        t16(inputs["Wq_re"] + inputs["Wq_im"], sc),
        t16(inputs["Wk_re"], sc), t16(inputs["Wk_im"], sc),
        t16(inputs["Wk_re"] + inputs["Wk_im"], sc),
        t16(inputs["Wv_re"]), t16(inputs["Wv_im"]),
        t16(inputs["Wv_re"] + inputs["Wv_im"]),
    ])
    bqk = np.stack([
        inputs["bq_re"] * sc, (inputs["bq_re"] + inputs["bq_im"]) * sc,
        inputs["bk_re"] * sc, (inputs["bk_re"] + inputs["bk_im"]) * sc,
    ]).astype(np.float32)                       # [4, 512]
    bqk = bqk.reshape(4, KC, P).transpose(2, 0, 1).copy()  # [128, 4, KC]

    xrT = inputs["x_re"].transpose(0, 2, 1).astype(np.float16)  # [B, D, N]
    xiT = inputs["x_im"].transpose(0, 2, 1).astype(np.float16)
    return [
        {
            "xrT": np.ascontiguousarray(xrT[c]),
            "xiT": np.ascontiguousarray(xiT[c]),
            "w9": w9,
            "bqk": bqk,
        }
        for c in range(NCORES)
    ]


_NC_CACHE = None


def get_nc():
    global _NC_CACHE
    if _NC_CACHE is None:
        _NC_CACHE = build_nc()
    return _NC_CACHE


def kernel(**inputs) -> np.ndarray:
    nc = get_nc()
    in_maps = make_in_maps(inputs)
    res = run_bass_kernel_spmd(nc, in_maps, core_ids=list(range(NCORES)))
    out = np.stack([res.results[c]["out"] for c in range(NCORES)], axis=1)
    out = out.astype(np.float32)
    out[0] += inputs["bv_re"].astype(np.float32)
    out[1] += inputs["bv_im"].astype(np.float32)
    return out



# revision 3
# speedup vs baseline: 1.0411x; 1.0411x over previous
"""Complex self-attention (single-head) on 8 Trainium2 NeuronCores.

Problem: y = stack(re, im) of softmax(|q k^T|/sqrt(D)) @ v with complex
q/k/v projections of a complex input x.  B=8, N=1024, D=512, fp32 I/O.

Strategy
--------
Data-parallel over the batch: core c computes batch c entirely locally.

Key tricks (vs a straightforward implementation):
  * k is never computed: s = q k^T = x~ (Wq~^T Wk~) x~^T with x~ = [x, 1]
    and W~ = [W | b].  The host precomputes M~ = Wq~^T Wk~ (complex,
    with the 1/sqrt(D) scale folded in).  Only y = x~ M~ (a q-like
    projection) is computed on device; scores are y x^T plus a rank-1
    correction u[n] = (x~ M~)[:, last-col], applied via a 1-partition
    fp8 DoubleRow matmul with a constant-64 lhsT.
  * All projection + score matmuls run in fp8 e4m3 DoubleRow mode
    (2 contraction planes per instruction at 0.5 cycles/row = 4x fp16
    throughput).  Precision is recovered with hi/lo residual splits:
    A@B ~ Ah@Bh + Ah@Bl + Al@Bh, giving ~11-bit effective mantissa.
    Host-side tensors (x, M~, Wv, u) are split on the host for free.
  * Everything is pre-scaled by powers of 2 (x: 2^4, M~: 2^9, Wv: 2^5,
    u: 2^5, y: 2^7, v: 2^2) so fp8 ranges are well used; the exact
    compensations fold into existing ACT scale slots (sqrt, casts).
  * Softmax needs NO max-reduction and NO transpose: scores are built
    transposed s^T[m, n] (m = key, on partitions), exp(|s| - 11) is safe
    in fp16 (max |s| ~ 15.8, every row's max is >= 8.6), the shift
    cancels in U/Z.  Z comes from a ones-column matmul (value 4.0 to
    cancel v's 2^2 scale).  att @ v runs in fp16 (E16, v16).
  * v bias is added on the host (exact since softmax rows sum to 1).
"""

from contextlib import ExitStack

import numpy as np
import ml_dtypes

import concourse.bass as bass
import concourse.mybir as mybir
import concourse.tile as tile
from concourse import bacc
from concourse.bass_utils import run_bass_kernel_spmd

B, N, D = 8, 1024, 512
P = 128
KC = D // P          # 4 contraction chunks of 128 (2 DoubleRow pairs)
MC = N // P          # 8 row chunks of 128
NH = 2               # halves of N (free dim <= 512 per matmul)
NCORES = 8
EXP_SHIFT = 11.0

F8NP = ml_dtypes.float8_e4m3
f8 = mybir.dt.float8e4
f16 = mybir.dt.float16
f32 = mybir.dt.float32
AF = mybir.ActivationFunctionType
ALU = mybir.AluOpType
DR = mybir.MatmulPerfMode.DoubleRow

# x8 slots
XRH, XRL, XIH, XIL, XNH, XNL, XSH, XSL = range(8)   # xn = -xi
# m8 / wv8 slots
WRH, WRL, WIH, WIL, WSH, WSL = range(6)
# y8 slots
YRH, YRL, YIH, YIL = range(4)


def emit(tc, ctx, nc, x8_d, m8_d, wv8_d, bias_d, u8_d, out_d):
    singles = ctx.enter_context(tc.tile_pool(name="singles", bufs=1))
    ps = ctx.enter_context(tc.tile_pool(name="ps", bufs=7, space="PSUM"))
    psz = ctx.enter_context(tc.tile_pool(name="psz", bufs=1, space="PSUM"))
    t1p = ctx.enter_context(tc.tile_pool(name="t1p", bufs=9))
    t12p = ctx.enter_context(tc.tile_pool(name="t12p", bufs=9))
    tmp = ctx.enter_context(tc.tile_pool(name="tmp", bufs=6))
    outp = ctx.enter_context(tc.tile_pool(name="outp", bufs=4))

    # ---- SBUF tensors -------------------------------------------------
    x8_sb = singles.tile([P, 8, KC, N], f8)
    m8_sb = singles.tile([P, 6, KC, D], f8)
    wv8_sb = singles.tile([P, 6, KC, D], f8)
    bias_sb = singles.tile([P, 2, KC], f32)
    u8_sb = singles.tile([1, 4, N], f8)
    y16_sb = singles.tile([P, 2, KC, N], f16)     # yr16 / yi16raw (2^7)
    y8_sb = singles.tile([P, 4, KC, N], f8)       # yr_h/l, yi_h/l
    e16_sb = singles.tile([P, MC, N], f16)        # E^T[m, n]
    v16_sb = singles.tile([P, 2, MC, D], f16)     # v re/im (2^2)

    # ---- input DMAs in first-use order (sync HWDGE queue) -------------
    x8_r = x8_d.rearrange("s (c p) n -> p s c n", p=P)
    m8_r = m8_d.rearrange("s (c p) e -> p s c e", p=P)
    wv8_r = wv8_d.rearrange("s (c p) e -> p s c e", p=P)
    nc.sync.dma_start(out=bias_sb, in_=bias_d)
    nc.sync.dma_start(out=u8_sb, in_=u8_d)
    for kc in range(KC):
        for s in (WRH, WRL):
            nc.sync.dma_start(out=m8_sb[:, s, kc], in_=m8_r[:, s, kc])
        for s in (XRH, XRL):
            nc.sync.dma_start(out=x8_sb[:, s, kc], in_=x8_r[:, s, kc])
    for kc in range(KC):
        for s in (WIH, WIL):
            nc.sync.dma_start(out=m8_sb[:, s, kc], in_=m8_r[:, s, kc])
        for s in (XIH, XIL):
            nc.sync.dma_start(out=x8_sb[:, s, kc], in_=x8_r[:, s, kc])
    for kc in range(KC):
        for s in (WSH, WSL):
            nc.sync.dma_start(out=m8_sb[:, s, kc], in_=m8_r[:, s, kc])
        for s in (XSH, XSL):
            nc.sync.dma_start(out=x8_sb[:, s, kc], in_=x8_r[:, s, kc])
    for kc in range(KC):
        for s in range(6):
            nc.sync.dma_start(out=wv8_sb[:, s, kc], in_=wv8_r[:, s, kc])
        for s in (XNH, XNL):
            nc.sync.dma_start(out=x8_sb[:, s, kc], in_=x8_r[:, s, kc])

    ones8 = singles.tile([1, 2, P], f8)
    nc.vector.memset(ones8, 64.0)
    ones_m = singles.tile([P, 1], f16)
    nc.vector.memset(ones_m, 4.0)
    shift_sb = singles.tile([P, 1], f32)
    nc.vector.memset(shift_sb, -EXP_SHIFT)

    def hl(pt, a, sa, acols, b, sb, bcols, first, last):
        """Accumulate (Ah+Al)@(Bh+Bl) - Al@Bl into pt: 6 DoubleRow matmuls.
        sa/sb are (hi, lo) slot pairs in a/b's slot dims."""
        n = 0
        for pa, pb in ((sa[0], sb[0]), (sa[0], sb[1]), (sa[1], sb[0])):
            for kp in range(2):
                nc.tensor.matmul(
                    pt,
                    lhsT=a[:, pa, 2 * kp:2 * kp + 2, acols],
                    rhs=b[:, pb, 2 * kp:2 * kp + 2, bcols],
                    start=(first and n == 0),
                    stop=(last and n == 5),
                    perf_mode=DR,
                )
                n += 1

    # ---- y = x~ M~ projection, Karatsuba, per half --------------------
    # yr = (P1 - P2)*2^-6 + br_r7;  yi_raw = (P3 - P1 - P2)*2^-6 - br_r7
    # (the +br_r7+br_i7 for yi is folded into the yi8 cast bias)
    def y_half(nh):
        ncols = slice(nh * 512, nh * 512 + 512)
        t1bs, t12s = {}, {}
        pts = {}
        for ec in range(KC):
            pt = ps.tile([P, 512], f32, tag="b", name="pt")
            hl(pt, m8_sb, (WRH, WRL), slice(ec * P, ec * P + P),
               x8_sb, (XRH, XRL), ncols, True, True)
            pts[ec] = pt
        for ec in range(KC):
            t1b = t1p.tile([P, 512], f32, tag="t1", name="t1b")
            nc.scalar.activation(out=t1b, in_=pts[ec], func=AF.Identity,
                                 scale=2.0**-6, bias=bias_sb[:, 0, ec:ec + 1])
            t1bs[ec] = t1b
        for ec in range(KC):
            pt = ps.tile([P, 512], f32, tag="b", name="pt")
            hl(pt, m8_sb, (WIH, WIL), slice(ec * P, ec * P + P),
               x8_sb, (XIH, XIL), ncols, True, True)
            pts[ec] = pt
        for ec in range(KC):
            nc.vector.scalar_tensor_tensor(
                out=y16_sb[:, 0, ec, ncols], in0=pts[ec], scalar=-(2.0**-6),
                in1=t1bs[ec], op0=ALU.mult, op1=ALU.add)
            t12b = t12p.tile([P, 512], f32, tag="t12", name="t12b")
            nc.vector.scalar_tensor_tensor(
                out=t12b, in0=pts[ec], scalar=2.0**-6,
                in1=t1bs[ec], op0=ALU.mult, op1=ALU.add)
            t12s[ec] = t12b
            # yr split: hi on ACT, lo on Pool (SBUF-only engine)
            nc.scalar.activation(out=y8_sb[:, YRH, ec, ncols],
                                 in_=y16_sb[:, 0, ec, ncols], func=AF.Copy)
            nc.gpsimd.tensor_sub(y8_sb[:, YRL, ec, ncols],
                                 y16_sb[:, 0, ec, ncols],
                                 y8_sb[:, YRH, ec, ncols])
        for ec in range(KC):
            pt = ps.tile([P, 512], f32, tag="b", name="pt")
            hl(pt, m8_sb, (WSH, WSL), slice(ec * P, ec * P + P),
               x8_sb, (XSH, XSL), ncols, True, True)
            pts[ec] = pt
        for ec in range(KC):
            nc.vector.scalar_tensor_tensor(
                out=y16_sb[:, 1, ec, ncols], in0=pts[ec], scalar=2.0**-6,
                in1=t12s[ec], op0=ALU.mult, op1=ALU.subtract)
            # yi = yi_raw + (br_r7 + br_i7): fold bias into casts
            nc.scalar.activation(out=y8_sb[:, YIH, ec, ncols],
                                 in_=y16_sb[:, 1, ec, ncols], func=AF.Identity,
                                 bias=bias_sb[:, 1, ec:ec + 1])
            nc.vector.scalar_tensor_tensor(
                out=y8_sb[:, YIL, ec, ncols], in0=y16_sb[:, 1, ec, ncols],
                scalar=bias_sb[:, 1, ec:ec + 1],
                in1=y8_sb[:, YIH, ec, ncols], op0=ALU.add, op1=ALU.subtract)

    # ---- v projection, Karatsuba, phase-major over all 8 m-chunks -----
    def v_proj():
        t1vs, t12vs, pts = {}, {}, {}
        for mc in range(MC):
            pt = ps.tile([P, 512], f32, tag="b", name="pt")
            hl(pt, x8_sb, (XRH, XRL), slice(mc * P, mc * P + P),
               wv8_sb, (WRH, WRL), slice(0, D), True, True)
            pts[mc] = pt
        for mc in range(MC):
            t1v = t1p.tile([P, 512], f32, tag="t1", name="t1v")
            nc.scalar.activation(out=t1v, in_=pts[mc], func=AF.Identity,
                                 scale=2.0**-7)
            t1vs[mc] = t1v
        for mc in range(MC):
            pt = ps.tile([P, 512], f32, tag="b", name="pt")
            hl(pt, x8_sb, (XIH, XIL), slice(mc * P, mc * P + P),
               wv8_sb, (WIH, WIL), slice(0, D), True, True)
            pts[mc] = pt
        for mc in range(MC):
            nc.vector.scalar_tensor_tensor(
                out=v16_sb[:, 0, mc], in0=pts[mc], scalar=-(2.0**-7),
                in1=t1vs[mc], op0=ALU.mult, op1=ALU.add)
            t12v = t12p.tile([P, 512], f32, tag="t12", name="t12v")
            nc.vector.scalar_tensor_tensor(
                out=t12v, in0=pts[mc], scalar=2.0**-7,
                in1=t1vs[mc], op0=ALU.mult, op1=ALU.add)
            t12vs[mc] = t12v
        for mc in range(MC):
            pt = ps.tile([P, 512], f32, tag="b", name="pt")
            hl(pt, x8_sb, (XSH, XSL), slice(mc * P, mc * P + P),
               wv8_sb, (WSH, WSL), slice(0, D), True, True)
            pts[mc] = pt
        for mc in range(MC):
            nc.vector.scalar_tensor_tensor(
                out=v16_sb[:, 1, mc], in0=pts[mc], scalar=2.0**-7,
                in1=t12vs[mc], op0=ALU.mult, op1=ALU.subtract)

    # ---- scores + |s| + exp, one tile [m-chunk, n-half] ---------------
    def score_tile(mc, nh):
        ncols = slice(nh * 512, nh * 512 + 512)
        mcols = slice(mc * P, mc * P + P)
        rt = ps.tile([P, 512], f32, tag="b", name="rt")
        hl(rt, x8_sb, (XRH, XRL), mcols, y8_sb, (YRH, YRL), ncols,
           True, False)
        hl(rt, x8_sb, (XNH, XNL), mcols, y8_sb, (YIH, YIL), ncols,
           False, False)
        nc.tensor.matmul(rt, lhsT=ones8, rhs=u8_sb[0:1, 0:2, ncols],
                         start=False, stop=True, perf_mode=DR)
        it = ps.tile([P, 512], f32, tag="b", name="it")
        hl(it, x8_sb, (XIH, XIL), mcols, y8_sb, (YRH, YRL), ncols,
           True, False)
        hl(it, x8_sb, (XRH, XRL), mcols, y8_sb, (YIH, YIL), ncols,
           False, False)
        nc.tensor.matmul(it, lhsT=ones8, rhs=u8_sb[0:1, 2:4, ncols],
                         start=False, stop=True, perf_mode=DR)
        t1 = tmp.tile([P, 512], f32, tag="sq", name="t1")
        nc.scalar.activation(out=t1, in_=rt, func=AF.Square)
        t2 = tmp.tile([P, 512], f32, tag="sq", name="t2")
        nc.scalar.activation(out=t2, in_=it, func=AF.Square)
        u = tmp.tile([P, 512], f32, tag="u", name="u")
        nc.gpsimd.tensor_add(u, t1, t2)
        a = tmp.tile([P, 512], f32, tag="a", name="a")
        nc.scalar.activation(out=a, in_=u, func=AF.Sqrt, scale=2.0**-22)
        nc.scalar.activation(out=e16_sb[:, mc, ncols], in_=a, func=AF.Exp,
                             bias=shift_sb)

    # ---- AV for one 128-wide output row block g -----------------------
    def av_group(g):
        zp = psz.tile([P, 1], f32, tag="z", name="zp")
        for mc in range(MC):
            nc.tensor.matmul(zp, lhsT=e16_sb[:, mc, g * P:(g + 1) * P],
                             rhs=ones_m, start=mc == 0, stop=mc == MC - 1)
        zr = tmp.tile([P, 1], f32, tag="zr", name="zr")
        nc.vector.reciprocal(zr, zp)
        ur = ps.tile([P, 512], f32, tag="b", name="ur")
        ui = ps.tile([P, 512], f32, tag="b", name="ui")
        for mc in range(MC):
            lh = e16_sb[:, mc, g * P:(g + 1) * P]
            st, sp = mc == 0, mc == MC - 1
            nc.tensor.matmul(ur, lhsT=lh, rhs=v16_sb[:, 0, mc],
                             start=st, stop=sp)
            nc.tensor.matmul(ui, lhsT=lh, rhs=v16_sb[:, 1, mc],
                             start=st, stop=sp)
        o0 = outp.tile([P, 512], f16, tag="o", name="o0")
        nc.vector.tensor_scalar_mul(o0, ur, zr)
        nc.sync.dma_start(out=out_d[0, g * P:(g + 1) * P], in_=o0)
        o1 = outp.tile([P, 512], f16, tag="o", name="o1")
        nc.scalar.activation(out=o1, in_=ui, func=AF.Copy, scale=zr)
        nc.scalar.dma_start(out=out_d[1, g * P:(g + 1) * P], in_=o1)

    # ---- schedule -----------------------------------------------------
    y_half(0)
    y_half(1)
    v_proj()
    for mc in range(MC):
        score_tile(mc, 0)
    score_tile(0, 1)
    score_tile(1, 1)
    av_group(0)
    av_group(1)
    av_group(2)
    for mc in range(2, MC):
        score_tile(mc, 1)
    av_group(3)
    for g in range(4, 8):
        av_group(g)


def build_nc():
    nc = bacc.Bacc("TRN2", target_bir_lowering=False, debug=False)
    x8_d = nc.dram_tensor("x8", [8, D, N], f8, kind="ExternalInput").ap()
    m8_d = nc.dram_tensor("m8", [6, D, D], f8, kind="ExternalInput").ap()
    wv8_d = nc.dram_tensor("wv8", [6, D, D], f8, kind="ExternalInput").ap()
    bias_d = nc.dram_tensor("biasx", [P, 2, KC], f32, kind="ExternalInput").ap()
    u8_d = nc.dram_tensor("u8", [1, 4, N], f8, kind="ExternalInput").ap()
    out_d = nc.dram_tensor("out", [2, N, D], f16, kind="ExternalOutput").ap()
    with tile.TileContext(nc) as tc, ExitStack() as ctx:
        emit(tc, ctx, nc, x8_d, m8_d, wv8_d, bias_d, u8_d, out_d)
    nc.compile()
    return nc


def _split8(a):
    h = a.astype(F8NP)
    l = (a - h.astype(np.float32)).astype(F8NP)
    return h, l


def _chunked(a):
    """[D, cols] -> [D, cols] laid out as dram [d, cols] (kept flat; the
    kernel rearranges (c p) itself)."""
    return np.ascontiguousarray(a)


def make_in_maps(inputs):
    SC = np.float64(1.0 / np.sqrt(D))
    Wq = (inputs["Wq_re"] + 1j * inputs["Wq_im"]).astype(np.complex128)
    Wk = (inputs["Wk_re"] + 1j * inputs["Wk_im"]).astype(np.complex128)
    bq = (inputs["bq_re"] + 1j * inputs["bq_im"]).astype(np.complex128)
    bk = (inputs["bk_re"] + 1j * inputs["bk_im"]).astype(np.complex128)
    Wq_ext = np.concatenate([Wq, bq[:, None]], axis=1)   # [e, d~]
    Wk_ext = np.concatenate([Wk, bk[:, None]], axis=1)
    Mt = (Wq_ext.astype(np.complex64).T @ Wk_ext.astype(np.complex64))
    Mt = Mt.astype(np.complex128) * SC                   # [d~, e~]
    M_hat = Mt[:D, :D]
    bias_row = Mt[D, :D]
    m_col = Mt[:D, D]
    corner = Mt[D, D]

    # M' = M_hat * 2^9, slots (r_h, r_l, i_h, i_l, s_h, s_l)
    def hl6(mat_r, mat_i):
        s_h, s_l = _split8((mat_r + mat_i).astype(np.float32))
        r_h, r_l = _split8(mat_r.astype(np.float32))
        i_h, i_l = _split8(mat_i.astype(np.float32))
        return np.stack([r_h, r_l, i_h, i_l, s_h, s_l])

    m8 = hl6(M_hat.real * 2.0**9, M_hat.imag * 2.0**9)
    wv8 = hl6(inputs["Wv_re"].T.astype(np.float64) * 2.0**5,
              inputs["Wv_im"].T.astype(np.float64) * 2.0**5)

    br_r7 = (bias_row.real * 2.0**7).astype(np.float32)
    br_i7 = (bias_row.imag * 2.0**7).astype(np.float32)
    biasx = np.stack([br_r7, br_r7 + br_i7])             # [2, D]
    biasx = biasx.reshape(2, KC, P).transpose(2, 0, 1).copy()  # [P, 2, KC]

    in_maps = []
    for c in range(NCORES):
        xr = inputs["x_re"][c].astype(np.float64)        # [N, D]
        xi = inputs["x_im"][c].astype(np.float64)
        x_c = xr + 1j * xi
        u = x_c @ m_col + corner                          # [N]
        u5 = (u * 2.0**5)
        ur_h, ur_l = _split8(u5.real.astype(np.float32))
        ui_h, ui_l = _split8(u5.imag.astype(np.float32))
        u8 = np.stack([ur_h, ur_l, ui_h, ui_l])[None]     # [1, 4, N]

        xT_r = (xr.T * 16.0).astype(np.float32)           # [D, N]
        xT_i = (xi.T * 16.0).astype(np.float32)
        xT_s = xT_r + xT_i
        xr_h, xr_l = _split8(xT_r)
        xi_h, xi_l = _split8(xT_i)
        xs_h, xs_l = _split8(xT_s)
        xn_h = -xi_h
        xn_l = -xi_l
        x8 = np.stack([xr_h, xr_l, xi_h, xi_l, xn_h, xn_l, xs_h, xs_l])

        in_maps.append({
            "x8": x8, "m8": m8, "wv8": wv8, "biasx": biasx, "u8": u8,
        })
    return in_maps


_NC_CACHE = None


def get_nc():
    global _NC_CACHE
    if _NC_CACHE is None:
        _NC_CACHE = build_nc()
    return _NC_CACHE


def kernel(**inputs) -> np.ndarray:
    nc = get_nc()
    in_maps = make_in_maps(inputs)
    res = run_bass_kernel_spmd(nc, in_maps, core_ids=list(range(NCORES)))
    out = np.stack([res.results[c]["out"] for c in range(NCORES)], axis=1)
    out = out.astype(np.float32)
    out[0] += inputs["bv_re"].astype(np.float32)
    out[1] += inputs["bv_im"].astype(np.float32)
    return out


# revision 4
# speedup vs baseline: 1.1945x; 1.1474x over previous
"""Complex self-attention (single-head) on 8 Trainium2 NeuronCores.

Problem: y = stack(re, im) of softmax(|q k^T|/sqrt(D)) @ v with complex
q/k/v projections of a complex input x.  B=8, N=1024, D=512, fp32 I/O.

Strategy
--------
Data-parallel over the batch: core c computes batch c entirely locally.

Key tricks (vs a straightforward implementation):
  * k is never computed: s = q k^T = x~ (Wq~^T Wk~) x~^T with x~ = [x, 1]
    and W~ = [W | b].  The host precomputes M~ = Wq~^T Wk~ (complex,
    1/sqrt(D) folded in).  Only y = x M^ (a q-like projection) runs on
    device; scores are y x^T plus a rank-1 column term u[n] (host
    matvec), applied via a 1-partition fp8 DoubleRow matmul against a
    constant-64 lhsT, and a rank-1 row term folded into the Square
    activation's per-partition bias (host matvec as well).
  * All projection + score matmuls run in fp8 e4m3 DoubleRow mode
    (2 contraction planes per instruction at 0.5 cycles/row = 4x fp16
    throughput).  Precision is recovered with hi/lo residual splits:
    A@B ~ Ah@Bh + Ah@Bl + Al@Bh (~11-bit effective mantissa).  All
    host-side operands (x, M~, Wv, u) are split on the host for free.
  * Power-of-2 pre-scales (x 2^4, M~ 2^9, Wv 2^5, u 2^5, y 2^7, v 2^2)
    keep fp8/fp16 ranges healthy; compensations fold into existing
    ACT scale slots (Square scale 2^-11 makes |s|^2 fit fp16).
  * Softmax needs NO max-reduction and NO transpose: scores land
    transposed s^T[m, n], exp(|s| - 11) is fp16-safe (max |s| ~ 15.8,
    row maxes >= 8.6), the shift cancels in U/Z.  Z via ones-column
    matmul (value 4.0 cancels v's 2^2 scale).  att @ v runs in fp16.
  * ACT function-table thrash is avoided by batching all Sqrt ops and
    all Exp ops per score half (sqrt and exp share no hw act-table
    set; Square/Identity/Copy are in every set).
  * v bias is added on the host (exact since softmax rows sum to 1).
"""

from contextlib import ExitStack

import numpy as np
import ml_dtypes

import concourse.bass as bass
import concourse.mybir as mybir
import concourse.tile as tile
from concourse import bacc
from concourse.bass_utils import run_bass_kernel_spmd

B, N, D = 8, 1024, 512
P = 128
KC = D // P          # 4 contraction chunks of 128 (2 DoubleRow pairs)
MC = N // P          # 8 row chunks of 128
NCORES = 8
EXP_SHIFT = 11.0

F8NP = ml_dtypes.float8_e4m3
f8 = mybir.dt.float8e4
f16 = mybir.dt.float16
f32 = mybir.dt.float32
AF = mybir.ActivationFunctionType
ALU = mybir.AluOpType
DR = mybir.MatmulPerfMode.DoubleRow

# x8 slots
XRH, XRL, XIH, XIL, XNH, XNL, XSH, XSL = range(8)   # xn = -xi
# m8 / wv8 slots
WRH, WRL, WIH, WIL, WSH, WSL = range(6)
# y8 slots
YRH, YRL, YIH, YIL = range(4)


def emit(tc, ctx, nc, x8_d, m8_d, wv8_d, bias_d, u8_d, wsq_d, out_d):
    singles = ctx.enter_context(tc.tile_pool(name="singles", bufs=1))
    ps = ctx.enter_context(tc.tile_pool(name="ps", bufs=7, space="PSUM"))
    psz = ctx.enter_context(tc.tile_pool(name="psz", bufs=1, space="PSUM"))
    t1p = ctx.enter_context(tc.tile_pool(name="t1p", bufs=9))
    t12p = ctx.enter_context(tc.tile_pool(name="t12p", bufs=9))
    sqp = ctx.enter_context(tc.tile_pool(name="sqp", bufs=4))
    up = ctx.enter_context(tc.tile_pool(name="up", bufs=9))
    ap_ = ctx.enter_context(tc.tile_pool(name="ap", bufs=9))
    tmp = ctx.enter_context(tc.tile_pool(name="tmp", bufs=4))
    outp = ctx.enter_context(tc.tile_pool(name="outp", bufs=6))

    # ---- SBUF tensors -------------------------------------------------
    x8_sb = singles.tile([P, 8, KC, N], f8)
    m8_sb = singles.tile([P, 6, KC, D], f8)
    wv8_sb = singles.tile([P, 6, KC, D], f8)
    bias_sb = singles.tile([P, KC], f32)
    u8_sb = singles.tile([1, 4, N], f8)
    wsq_sb = singles.tile([P, 2, MC], f32)
    y16_sb = singles.tile([P, 2, KC, N], f16)     # yr16 / yi16raw (2^7)
    y8_sb = singles.tile([P, 4, KC, N], f8)       # yr_h/l, yi_h/l
    e16_sb = singles.tile([P, MC, N], f16)        # E^T[m, n]
    v16_sb = singles.tile([P, 2, MC, D], f16)     # v re/im (2^2)

    # ---- input DMAs, first-use order (sync HWDGE queue) ---------------
    # Fine-grained for the very first product (y h0 phase A); whole-slot
    # transfers for everything else to keep HWDGE occupancy low.
    x8_r = x8_d.rearrange("s (c p) n -> p s c n", p=P)
    m8_r = m8_d.rearrange("s (c p) e -> p s c e", p=P)
    wv8_r = wv8_d.rearrange("s (c p) e -> p s c e", p=P)
    for kc in (0, 1):
        nc.sync.dma_start(out=m8_sb[:, WRH, kc], in_=m8_r[:, WRH, kc])
    for kc in (0, 1):
        nc.sync.dma_start(out=x8_sb[:, XRH, kc], in_=x8_r[:, XRH, kc])
    for kc in (2, 3):
        nc.sync.dma_start(out=m8_sb[:, WRH, kc], in_=m8_r[:, WRH, kc])
    for kc in (2, 3):
        nc.sync.dma_start(out=x8_sb[:, XRH, kc], in_=x8_r[:, XRH, kc])
    for kc in range(KC):
        nc.sync.dma_start(out=x8_sb[:, XRL, kc], in_=x8_r[:, XRL, kc])
    for kc in range(KC):
        nc.sync.dma_start(out=m8_sb[:, WRL, kc], in_=m8_r[:, WRL, kc])
    nc.sync.dma_start(out=bias_sb, in_=bias_d)
    nc.sync.dma_start(out=u8_sb, in_=u8_d)
    nc.sync.dma_start(out=wsq_sb, in_=wsq_d)
    for s in (WIH, WIL):
        nc.sync.dma_start(out=m8_sb[:, s], in_=m8_r[:, s])
    for s in (XIH, XIL):
        nc.sync.dma_start(out=x8_sb[:, s], in_=x8_r[:, s])
    for s in (WSH, WSL):
        nc.sync.dma_start(out=m8_sb[:, s], in_=m8_r[:, s])
    for s in (XSH, XSL):
        nc.sync.dma_start(out=x8_sb[:, s], in_=x8_r[:, s])
    for s in range(6):
        nc.sync.dma_start(out=wv8_sb[:, s], in_=wv8_r[:, s])
    for s in (XNH, XNL):
        nc.sync.dma_start(out=x8_sb[:, s], in_=x8_r[:, s])

    ones8 = singles.tile([1, 2, P], f8)
    nc.vector.memset(ones8, 64.0)
    ones_m = singles.tile([P, 1], f16)
    nc.vector.memset(ones_m, 4.0)
    shift_sb = singles.tile([P, 1], f32)
    nc.vector.memset(shift_sb, -EXP_SHIFT)

    def hl(pt, a, sa, acols, b, sb, bcols, first, last):
        """Accumulate (Ah+Al)@(Bh+Bl) - Al@Bl into pt: 6 DoubleRow matmuls."""
        n = 0
        for pa, pb in ((sa[0], sb[0]), (sa[0], sb[1]), (sa[1], sb[0])):
            for kp in range(2):
                nc.tensor.matmul(
                    pt,
                    lhsT=a[:, pa, 2 * kp:2 * kp + 2, acols],
                    rhs=b[:, pb, 2 * kp:2 * kp + 2, bcols],
                    start=(first and n == 0),
                    stop=(last and n == 5),
                    perf_mode=DR,
                )
                n += 1

    # ---- y = x~ M~ projection, Karatsuba, per half --------------------
    def y_half(nh):
        ncols = slice(nh * 512, nh * 512 + 512)
        t1bs, t12s = {}, {}
        pts = {}
        for ec in range(KC):
            pt = ps.tile([P, 512], f32, tag="b", name="pt")
            hl(pt, m8_sb, (WRH, WRL), slice(ec * P, ec * P + P),
               x8_sb, (XRH, XRL), ncols, True, True)
            pts[ec] = pt
        for ec in range(KC):
            t1b = t1p.tile([P, 512], f32, tag="t1", name="t1b")
            nc.vector.tensor_scalar(t1b, pts[ec], 2.0**-6,
                                    bias_sb[:, ec:ec + 1], ALU.mult, ALU.add)
            t1bs[ec] = t1b
        for ec in range(KC):
            pt = ps.tile([P, 512], f32, tag="b", name="pt")
            hl(pt, m8_sb, (WIH, WIL), slice(ec * P, ec * P + P),
               x8_sb, (XIH, XIL), ncols, True, True)
            pts[ec] = pt
        for ec in range(KC):
            nc.vector.scalar_tensor_tensor(
                out=y16_sb[:, 0, ec, ncols], in0=pts[ec], scalar=-(2.0**-6),
                in1=t1bs[ec], op0=ALU.mult, op1=ALU.add)
            t12b = t12p.tile([P, 512], f32, tag="t12", name="t12b")
            nc.vector.scalar_tensor_tensor(
                out=t12b, in0=pts[ec], scalar=2.0**-6,
                in1=t1bs[ec], op0=ALU.mult, op1=ALU.add)
            t12s[ec] = t12b
            nc.gpsimd.tensor_copy(y8_sb[:, YRH, ec, ncols],
                                  y16_sb[:, 0, ec, ncols])
            nc.gpsimd.tensor_sub(y8_sb[:, YRL, ec, ncols],
                                 y16_sb[:, 0, ec, ncols],
                                 y8_sb[:, YRH, ec, ncols])
        for ec in range(KC):
            pt = ps.tile([P, 512], f32, tag="b", name="pt")
            hl(pt, m8_sb, (WSH, WSL), slice(ec * P, ec * P + P),
               x8_sb, (XSH, XSL), ncols, True, True)
            pts[ec] = pt
        for ec in range(KC):
            nc.vector.scalar_tensor_tensor(
                out=y16_sb[:, 1, ec, ncols], in0=pts[ec], scalar=2.0**-6,
                in1=t12s[ec], op0=ALU.mult, op1=ALU.subtract)
            nc.gpsimd.tensor_copy(y8_sb[:, YIH, ec, ncols],
                                  y16_sb[:, 1, ec, ncols])
            nc.gpsimd.tensor_sub(y8_sb[:, YIL, ec, ncols],
                                 y16_sb[:, 1, ec, ncols],
                                 y8_sb[:, YIH, ec, ncols])

    # ---- v projection, Karatsuba, phase-major over all 8 m-chunks -----
    def v_proj():
        t1vs, t12vs, pts = {}, {}, {}
        for mc in range(MC):
            pt = ps.tile([P, 512], f32, tag="b", name="pt")
            hl(pt, x8_sb, (XRH, XRL), slice(mc * P, mc * P + P),
               wv8_sb, (WRH, WRL), slice(0, D), True, True)
            pts[mc] = pt
        for mc in range(MC):
            t1v = t1p.tile([P, 512], f32, tag="t1", name="t1v")
            nc.vector.tensor_scalar(t1v, pts[mc], 2.0**-7, None, ALU.mult)
            t1vs[mc] = t1v
        for mc in range(MC):
            pt = ps.tile([P, 512], f32, tag="b", name="pt")
            hl(pt, x8_sb, (XIH, XIL), slice(mc * P, mc * P + P),
               wv8_sb, (WIH, WIL), slice(0, D), True, True)
            pts[mc] = pt
        for mc in range(MC):
            nc.vector.scalar_tensor_tensor(
                out=v16_sb[:, 0, mc], in0=pts[mc], scalar=-(2.0**-7),
                in1=t1vs[mc], op0=ALU.mult, op1=ALU.add)
            t12v = t12p.tile([P, 512], f32, tag="t12", name="t12v")
            nc.vector.scalar_tensor_tensor(
                out=t12v, in0=pts[mc], scalar=2.0**-7,
                in1=t1vs[mc], op0=ALU.mult, op1=ALU.add)
            t12vs[mc] = t12v
        for mc in range(MC):
            pt = ps.tile([P, 512], f32, tag="b", name="pt")
            hl(pt, x8_sb, (XSH, XSL), slice(mc * P, mc * P + P),
               wv8_sb, (WSH, WSL), slice(0, D), True, True)
            pts[mc] = pt
        for mc in range(MC):
            nc.vector.scalar_tensor_tensor(
                out=v16_sb[:, 1, mc], in0=pts[mc], scalar=2.0**-7,
                in1=t12vs[mc], op0=ALU.mult, op1=ALU.subtract)

    # ---- scores: matmuls + squares (per tile), sqrt/exp batched -------
    us = {}

    def score_mm(mc, nh):
        ncols = slice(nh * 512, nh * 512 + 512)
        mcols = slice(mc * P, mc * P + P)
        rt = ps.tile([P, 512], f32, tag="b", name="rt")
        hl(rt, x8_sb, (XRH, XRL), mcols, y8_sb, (YRH, YRL), ncols,
           True, False)
        hl(rt, x8_sb, (XNH, XNL), mcols, y8_sb, (YIH, YIL), ncols,
           False, False)
        nc.tensor.matmul(rt, lhsT=ones8, rhs=u8_sb[0:1, 0:2, ncols],
                         start=False, stop=True, perf_mode=DR)
        it = ps.tile([P, 512], f32, tag="b", name="it")
        hl(it, x8_sb, (XIH, XIL), mcols, y8_sb, (YRH, YRL), ncols,
           True, False)
        hl(it, x8_sb, (XRH, XRL), mcols, y8_sb, (YIH, YIL), ncols,
           False, False)
        nc.tensor.matmul(it, lhsT=ones8, rhs=u8_sb[0:1, 2:4, ncols],
                         start=False, stop=True, perf_mode=DR)
        # (2^-11 s' + w)^2 so |s|^2 fits fp16; w = host rank-1 row term
        t1 = sqp.tile([P, 512], f16, tag="sq", name="t1")
        nc.scalar.activation(out=t1, in_=rt, func=AF.Square,
                             scale=2.0**-11, bias=wsq_sb[:, 0, mc:mc + 1])
        t2 = sqp.tile([P, 512], f16, tag="sq", name="t2")
        nc.scalar.activation(out=t2, in_=it, func=AF.Square,
                             scale=2.0**-11, bias=wsq_sb[:, 1, mc:mc + 1])
        u = up.tile([P, 512], f16, tag="u", name="u")
        nc.vector.tensor_add(u, t1, t2)
        us[mc, nh] = u

    def score_finish(nh):
        ncols = slice(nh * 512, nh * 512 + 512)
        sqs = {}
        for mc in range(MC):
            a = ap_.tile([P, 512], f16, tag="a", name="a")
            nc.scalar.activation(out=a, in_=us[mc, nh], func=AF.Sqrt)
            sqs[mc] = a
        for mc in range(MC):
            nc.scalar.activation(out=e16_sb[:, mc, ncols], in_=sqs[mc],
                                 func=AF.Exp, bias=shift_sb)

    # ---- AV for one 128-wide output row block g -----------------------
    def av_group(g, split=False):
        zp = psz.tile([P, 1], f32, tag="z", name="zp")
        for mc in range(MC):
            nc.tensor.matmul(zp, lhsT=e16_sb[:, mc, g * P:(g + 1) * P],
                             rhs=ones_m, start=mc == 0, stop=mc == MC - 1)
        zr = tmp.tile([P, 1], f32, tag="zr", name="zr")
        nc.vector.reciprocal(zr, zp)
        halves = ((0, 256), (256, 256)) if split else ((0, 512),)
        for d0, dw in halves:
            ur = ps.tile([P, 512], f32, tag="b", name="ur")
            ui = ps.tile([P, 512], f32, tag="b", name="ui")
            for mc in range(MC):
                lh = e16_sb[:, mc, g * P:(g + 1) * P]
                st, sp = mc == 0, mc == MC - 1
                nc.tensor.matmul(ur[:, :dw], lhsT=lh,
                                 rhs=v16_sb[:, 0, mc, d0:d0 + dw],
                                 start=st, stop=sp)
                nc.tensor.matmul(ui[:, :dw], lhsT=lh,
                                 rhs=v16_sb[:, 1, mc, d0:d0 + dw],
                                 start=st, stop=sp)
            o0 = outp.tile([P, 512], f16, tag="o", name="o0")
            nc.vector.tensor_scalar_mul(o0[:, :dw], ur[:, :dw], zr)
            nc.sync.dma_start(out=out_d[0, g * P:(g + 1) * P, d0:d0 + dw],
                              in_=o0[:, :dw])
            o1 = outp.tile([P, 512], f16, tag="o", name="o1")
            nc.scalar.activation(out=o1[:, :dw], in_=ui[:, :dw],
                                 func=AF.Copy, scale=zr)
            nc.scalar.dma_start(out=out_d[1, g * P:(g + 1) * P, d0:d0 + dw],
                                in_=o1[:, :dw])

    # ---- schedule -----------------------------------------------------
    y_half(0)
    y_half(1)
    v_proj()
    for mc in range(MC):
        score_mm(mc, 0)
    score_finish(0)
    for mc in range(MC):
        score_mm(mc, 1)
    score_finish(1)
    for g in range(7):
        av_group(g)
    av_group(7, split=True)


def build_nc():
    nc = bacc.Bacc("TRN2", target_bir_lowering=False, debug=False)
    x8_d = nc.dram_tensor("x8", [8, D, N], f8, kind="ExternalInput").ap()
    m8_d = nc.dram_tensor("m8", [6, D, D], f8, kind="ExternalInput").ap()
    wv8_d = nc.dram_tensor("wv8", [6, D, D], f8, kind="ExternalInput").ap()
    bias_d = nc.dram_tensor("biasx", [P, KC], f32, kind="ExternalInput").ap()
    u8_d = nc.dram_tensor("u8", [1, 4, N], f8, kind="ExternalInput").ap()
    wsq_d = nc.dram_tensor("wsq", [P, 2, MC], f32, kind="ExternalInput").ap()
    out_d = nc.dram_tensor("out", [2, N, D], f16, kind="ExternalOutput").ap()
    with tile.TileContext(nc) as tc, ExitStack() as ctx:
        emit(tc, ctx, nc, x8_d, m8_d, wv8_d, bias_d, u8_d, wsq_d, out_d)
    nc.compile()
    return nc


def _split8(a):
    h = a.astype(F8NP)
    l = (a - h.astype(np.float32)).astype(F8NP)
    return h, l


def make_in_maps(inputs):
    SC = np.float64(1.0 / np.sqrt(D))
    Wq = (inputs["Wq_re"] + 1j * inputs["Wq_im"]).astype(np.complex128)
    Wk = (inputs["Wk_re"] + 1j * inputs["Wk_im"]).astype(np.complex128)
    bq = (inputs["bq_re"] + 1j * inputs["bq_im"]).astype(np.complex128)
    bk = (inputs["bk_re"] + 1j * inputs["bk_im"]).astype(np.complex128)
    Wq_ext = np.concatenate([Wq, bq[:, None]], axis=1)   # [e, d~]
    Wk_ext = np.concatenate([Wk, bk[:, None]], axis=1)
    Mt = (Wq_ext.astype(np.complex64).T @ Wk_ext.astype(np.complex64))
    Mt = Mt.astype(np.complex128) * SC                   # [d~, e~]
    M_hat = Mt[:D, :D]
    bias_row = Mt[D, :D]
    m_col = Mt[:D, D]
    corner = Mt[D, D]

    def hl6(mat_r, mat_i):
        s_h, s_l = _split8((mat_r + mat_i).astype(np.float32))
        r_h, r_l = _split8(mat_r.astype(np.float32))
        i_h, i_l = _split8(mat_i.astype(np.float32))
        return np.stack([r_h, r_l, i_h, i_l, s_h, s_l])

    m8 = hl6(M_hat.real * 2.0**9, M_hat.imag * 2.0**9)
    wv8 = hl6(inputs["Wv_re"].T.astype(np.float64) * 2.0**5,
              inputs["Wv_im"].T.astype(np.float64) * 2.0**5)

    br_r7 = (bias_row.real * 2.0**7).astype(np.float32)
    biasx = br_r7.reshape(KC, P).T.copy()                # [P, KC]
    delta = (bias_row.real + bias_row.imag)              # natural scale [D]

    in_maps = []
    for c in range(NCORES):
        xr = inputs["x_re"][c].astype(np.float64)        # [N, D]
        xi = inputs["x_im"][c].astype(np.float64)
        x_c = xr + 1j * xi
        u = x_c @ m_col + corner                          # [N]
        u5 = u * 2.0**5
        ur_h, ur_l = _split8(u5.real.astype(np.float32))
        ui_h, ui_l = _split8(u5.imag.astype(np.float32))
        u8 = np.stack([ur_h, ur_l, ui_h, ui_l])[None]     # [1, 4, N]

        # rank-1 row corrections (for the missing yi bias), natural scale
        w_r = (-(xi @ delta)).astype(np.float32)          # [N]
        w_i = ((xr @ delta)).astype(np.float32)
        wsq = np.stack([w_r.reshape(MC, P), w_i.reshape(MC, P)])
        wsq = wsq.transpose(2, 0, 1).copy()               # [P, 2, MC]

        xT_r = (xr.T * 16.0).astype(np.float32)           # [D, N]
        xT_i = (xi.T * 16.0).astype(np.float32)
        xT_s = xT_r + xT_i
        xr_h, xr_l = _split8(xT_r)
        xi_h, xi_l = _split8(xT_i)
        xs_h, xs_l = _split8(xT_s)
        x8 = np.stack([xr_h, xr_l, xi_h, xi_l, -xi_h, -xi_l, xs_h, xs_l])

        in_maps.append({
            "x8": x8, "m8": m8, "wv8": wv8, "biasx": biasx, "u8": u8,
            "wsq": wsq,
        })
    return in_maps


_NC_CACHE = None


def get_nc():
    global _NC_CACHE
    if _NC_CACHE is None:
        _NC_CACHE = build_nc()
    return _NC_CACHE


def kernel(**inputs) -> np.ndarray:
    nc = get_nc()
    in_maps = make_in_maps(inputs)
    res = run_bass_kernel_spmd(nc, in_maps, core_ids=list(range(NCORES)))
    out = np.stack([res.results[c]["out"] for c in range(NCORES)], axis=1)
    out = out.astype(np.float32)
    out[0] += inputs["bv_re"].astype(np.float32)
    out[1] += inputs["bv_im"].astype(np.float32)
    return out
